# revision 1
# baseline (speedup 1.0000x reference)
"""Multi-head scaled-dot-product attention on 8 Trainium2 NeuronCores.

Problem: x[4,2048,128], Wq/Wk/Wv[10,128,128] (torch Linear layout [e_out,d_in]),
Wo[128,1280], bo[128]  ->  out[4,2048,128]

Sharding: 8 cores = 4 batches x 2 head-groups (5 heads each). Each core
computes its batch's attention for its 5 heads plus the partial output
projection; the host sums the two half-head partials per batch, transposes,
and adds the bias.

Per-core layout strategy (all host-side pre-transposed, so no on-chip
transposes at all):
  xT   [d=128, n=2048]  = x[b].T
  wq/wk/wv [5, d, e]    = W*.transpose(0,2,1)   (so lhsT = W*T directly)
  wo   [5, e, dout]     = Wo.T.reshape(10,128,128)[head slice]
  QT_h [e, n]  = wqT_h.T @ xT          (matmul lhsT=wq, rhs=xT)
  KT_h [e, n]  = wkT_h.T @ xT
  V_h  [m, e]  = xT_chunk.T @ wvT_h    (natural layout, m on partitions)
  ST   [m-chunk, nb] = KT_slice.T @ QT_slice   (scores transposed: keys on
       partitions -> softmax denominator via ones-matmul, P^T is directly
       what the PV matmul needs as rhs)
  PT   = exp(ST / sqrt(D))             (ACT, no max-subtraction needed:
       scores are ~N(0,1), |S|<~7, exp is safe and exact in fp32)
  OT_h [e, nb] += V_chunk.T @ PT_chunk (accumulated over 16 m-chunks)
  den  [1, nb] += ones.T @ PT_chunk
  OTn  = OT * broadcast(1/den)         (K=1 ones matmul broadcasts recip)
  outT [dout, nb] += wo_h.T @ OTn      (accumulated over 5 heads)
"""

from contextlib import ExitStack

import numpy as np

import concourse.tile as tile
from concourse import bacc, mybir
from concourse.bass import ds, ts
from concourse.bass_utils import run_bass_kernel_spmd

B, N, D, H = 4, 2048, 128, 10
HL = H // 2  # heads per core
NCHUNK = N // 128  # 16 key chunks
NBLK = N // 512  # 4 query blocks
INV_SCALE = float(1.0 / (128.0**0.5 + 1e-8))
f32 = mybir.dt.float32

PROFILE = False
LAST_RESULTS = None

_built = None


def _emit(tc, xT, xn, wq, wk, w2, ones_dram, outT):
    nc = tc.nc
    Exp = mybir.ActivationFunctionType.Exp
    fp16 = mybir.dt.float16

    def r(ap):
        return ap

    ctx = ExitStack()
    consts = ctx.enter_context(tc.tile_pool(name="consts", bufs=1))
    proj = ctx.enter_context(tc.tile_pool(name="proj", bufs=1))
    ps = ctx.enter_context(tc.tile_pool(name="ps", bufs=2, space="PSUM"))
    otps = ctx.enter_context(tc.tile_pool(name="otps", bufs=2, space="PSUM"))
    dnps = ctx.enter_context(tc.tile_pool(name="dnps", bufs=1, space="PSUM"))
    outps = ctx.enter_context(tc.tile_pool(name="outps", bufs=1, space="PSUM"))
    ptp = ctx.enter_context(tc.tile_pool(name="ptp", bufs=4))
    work = ctx.enter_context(tc.tile_pool(name="work", bufs=2))

    ones_mat = consts.tile([128, 128], fp16)
    xT_sb = consts.tile([D, N], fp16)
    xn_sb = consts.tile([D, N], fp16)  # chunk-major natural x: [p, c*128+d]
    wq_sb = consts.tile([D, HL * D], fp16)
    wk_sb = consts.tile([D, HL * D], fp16)
    w2_sb = consts.tile([D, HL * D], fp16)
    # head-0 weights and the first xT block first, so projections start early
    nc.sync.dma_start(wq_sb[:, ts(0, D)], wq[0])
    nc.sync.dma_start(wk_sb[:, ts(0, D)], wk[0])
    for j in range(NBLK):
        nc.sync.dma_start(xT_sb[:, ts(j, 512)], xT[:, ts(j, 512)])
    nc.gpsimd.dma_start(
        xn_sb[:].rearrange("p (c d) -> p c d", c=NCHUNK),
        xn.rearrange("(c p) d -> p c d", p=128),
    )
    nc.gpsimd.dma_start(ones_mat[:], ones_dram)
    for h in range(1, HL):
        nc.sync.dma_start(wq_sb[:, ts(h, D)], wq[h])
        nc.sync.dma_start(wk_sb[:, ts(h, D)], wk[h])
    for h in range(HL):
        nc.gpsimd.dma_start(w2_sb[:, ts(h, D)], w2[h])

    qt = proj.tile([D, HL * N], fp16)
    kt = proj.tile([D, HL * N], fp16)

    # ---- projections ----
    # During the projection phase the attention PSUM pools are idle; rotate
    # staging tiles through their tags so evacuation doesn't serialize on a
    # starved slot pool. Evacuations alternate between ScalarE and VectorE.
    proj_slots = [
        (ps, "st"),
        (otps, "ot_ps"),
        (ps, "st"),
        (dnps, "dn_ps"),
        (outps, "outp"),
    ]
    pctr = [0]

    def proj_tile(shape):
        pool, tag = proj_slots[pctr[0] % len(proj_slots)]
        pctr[0] += 1
        return pool.tile(shape, f32, tag=tag, name=f"proj{pctr[0]}")

    def proj_evac(dst, src):
        if pctr[0] % 2:
            nc.scalar.copy(dst, src)
        else:
            nc.vector.tensor_copy(dst, src)

    for h in range(HL):
        for j in range(NBLK):
            p = proj_tile([128, 512])
            nc.tensor.matmul(
                p[:],
                r(wq_sb[:, ts(h, D)]),
                r(xT_sb[:, ts(j, 512)]),
                start=True,
                stop=True,
            )
            proj_evac(qt[:, ds(h * N + j * 512, 512)], p[:])
        for j in range(NBLK):
            p = proj_tile([128, 512])
            nc.tensor.matmul(
                p[:],
                r(wk_sb[:, ts(h, D)]),
                r(xT_sb[:, ts(j, 512)]),
                start=True,
                stop=True,
            )
            proj_evac(kt[:, ds(h * N + j * 512, 512)], p[:])

    # ---- attention (software-pipelined emission) ----
    # pending epilogue state from the previous (nb, h)
    pend = None  # dict with ot_ps, recip, outp, h, is_last_head

    def emit_finish(st):
        otn = work.tile([128, 512], fp16, tag="otn")
        nc.vector.tensor_mul(otn[:], st["ot_ps"][:], st["bc"][:])
        nc.tensor.matmul(
            st["outp"][:],
            r(w2_sb[:, ts(st["h"], D)]),
            r(otn[:]),
            start=(st["h"] == 0),
            stop=(st["h"] == HL - 1),
        )
        if st["h"] == HL - 1:
            osb = work.tile([128, 512], f32, tag="osb")
            nc.vector.tensor_copy(osb[:], st["outp"][:])
            nc.sync.dma_start(outT[:, ts(st["nb"], 512)], osb[:])

    for nb in range(NBLK):
        outp = outps.tile([128, 512], f32)
        for h in range(HL):
            ot_ps = otps.tile([128, 512], f32)
            dn_ps = dnps.tile([128, 512], f32)
            # denominator: all pairs accumulate on DVE in fp16; PE reduces
            # the folded accumulator with two ones-matmuls at the end.
            acc = None

            def ot_den(pc, pp):
                nonlocal acc
                for j in range(2):
                    cc = 2 * pc + j
                    nc.tensor.matmul(
                        ot_ps[:],
                        xn_sb[:, ts(cc, 128)],
                        pp[:, j],
                        start=(cc == 0),
                        stop=(cc == NCHUNK - 1),
                    )
                if pc == 0:
                    acc = work.tile([128, 2, 512], fp16, tag="dacc")
                    nc.vector.tensor_copy(acc[:], pp[:])
                else:
                    nc.vector.tensor_add(acc[:], acc[:], pp[:])

            prev = None  # previous chunk-pair's PT tile
            for cp in range(NCHUNK // 2):
                stp = ps.tile([128, 2, 512], f32, tag="st")
                for j in range(2):
                    nc.tensor.matmul(
                        stp[:, j],
                        r(kt[:, ds(h * N + (2 * cp + j) * 128, 128)]),
                        r(qt[:, ds(h * N + nb * 512, 512)]),
                        start=True,
                        stop=True,
                    )
                p = ptp.tile([128, 2, 512], fp16, tag="pt")
                nc.scalar.activation(p[:], stp[:], Exp, scale=INV_SCALE)
                if prev is not None:
                    ot_den(*prev)
                prev = (cp, p)
                # interleave the previous head's epilogue into this head's
                # chunk stream so PE never waits on the DVE/DMA chain
                if pend is not None and cp == 5:
                    emit_finish(pend)
                    pend = None
            ot_den(*prev)
            for j in range(2):
                nc.tensor.matmul(
                    dn_ps[:],
                    ones_mat[:],
                    acc[:, j],
                    start=(j == 0),
                    stop=(j == 1),
                )
            bc = work.tile([128, 512], f32, tag="bc")
            nc.vector.reciprocal_approx_fast(out=bc[:], in_=dn_ps[:])
            pend = {
                "ot_ps": ot_ps,
                "bc": bc,
                "outp": outp,
                "h": h,
                "nb": nb,
            }
    # flush the last epilogue
    emit_finish(pend)
    pend = None
    ctx.close()


def _build():
    fp16 = mybir.dt.float16
    nc = bacc.Bacc("TRN2", target_bir_lowering=False, debug=False)
    xT = nc.dram_tensor("xT", [D, N], fp16, kind="ExternalInput").ap()
    wq = nc.dram_tensor("wq", [HL, D, D], fp16, kind="ExternalInput").ap()
    wk = nc.dram_tensor("wk", [HL, D, D], fp16, kind="ExternalInput").ap()
    xn = nc.dram_tensor("xn", [N, D], fp16, kind="ExternalInput").ap()
    w2 = nc.dram_tensor("w2", [HL, D, D], fp16, kind="ExternalInput").ap()
    ones_dram = nc.dram_tensor("ones", [D, D], fp16, kind="ExternalInput").ap()
    outT = nc.dram_tensor("outT", [D, N], f32, kind="ExternalOutput").ap()
    with tile.TileContext(nc) as tc:
        with nc.allow_low_precision(reason="float32r matmul operands (hi/lo rounding)"):
            _emit(tc, xT, xn, wq, wk, w2, ones_dram, outT)
    nc.compile()
    return nc


def kernel(x, Wq, Wk, Wv, Wo, bo):
    global _built, LAST_RESULTS
    x = np.asarray(x, dtype=np.float32)
    Wq = np.asarray(Wq, dtype=np.float32)
    Wk = np.asarray(Wk, dtype=np.float32)
    Wv = np.asarray(Wv, dtype=np.float32)
    Wo = np.asarray(Wo, dtype=np.float32)
    bo = np.asarray(bo, dtype=np.float32)

    if _built is None:
        _built = _build()
    nc = _built

    WqT = np.ascontiguousarray(Wq.transpose(0, 2, 1).astype(np.float16))
    WkT = np.ascontiguousarray(Wk.transpose(0, 2, 1).astype(np.float16))
    # fold the V projection into the output projection: W2_h = WvT_h @ WoT_h
    WvT = Wq.dtype.type(0)  # placeholder, unused
    W2 = np.ascontiguousarray(
        np.einsum(
            "hde,heo->hdo", Wv.transpose(0, 2, 1), Wo.T.reshape(H, D, D)
        ).astype(np.float16)
    )

    in_maps = []
    for c in range(8):
        b, g = divmod(c, 2)
        hsl = slice(g * HL, g * HL + HL)
        in_maps.append(
            {
                "xT": np.ascontiguousarray(x[b].T.astype(np.float16)),
                "wq": WqT[hsl],
                "wk": WkT[hsl],
                "xn": np.ascontiguousarray(x[b].astype(np.float16)),
                "w2": W2[hsl],
                "ones": np.ones((D, D), dtype=np.float16),
            }
        )

    res = run_bass_kernel_spmd(
        nc, in_maps, core_ids=list(range(8)), trace=PROFILE
    )
    LAST_RESULTS = res

    out = np.empty((B, N, D), dtype=np.float32)
    for b in range(B):
        oT = res.results[2 * b]["outT"] + res.results[2 * b + 1]["outT"]
        out[b] = oT.T
    out += bo
    return out



# revision 3
# speedup vs baseline: 1.1146x; 1.1146x over previous
"""Multi-head scaled-dot-product attention on 8 Trainium2 NeuronCores.

Problem: x[4,2048,128], Wq/Wk/Wv[10,128,128] (torch Linear layout [e_out,d_in]),
Wo[128,1280], bo[128]  ->  out[4,2048,128]

Sharding: 8 cores = 4 batches x 2 head-groups (5 heads each). Each core
computes its batch's attention for its 5 heads plus the partial output
projection; the host sums the two half-head partials per batch, transposes,
and adds the bias.

Math folding (host side, per head h):
  A_h  = Wq_h^T @ Wk_h          [D,D]   (one projection replaces Q and K:
         S = X Wq^T Wk X^T = G X^T with G = X A)
  W2_h = Wv_h^T @ Wo_h^T        [D,D]   (V-projection folded into out-proj:
         out_h = (P X) W2_h)

Per-core layout (all host-side pre-transposed; zero on-chip transposes):
  xT   [d,n]    = x[b].T            lhsT chunks for scores, rhs for proj
  xn   [p,c,d]  : xn[p,c,:] = x[c*128+p,:]   lhsT chunks for P@X
  gt   [d2,n]   = A^T X^T per head  (proj matmul: lhsT=A_h, rhs=xT block)
  ST   [k-chunk, q-blk] = xT_chunk.T @ gt_blk   (scores, keys on partitions)
  PT   = exp(ST / sqrt(D))          (ACT; scores ~N(0,1), exp safe in fp32)
  OT   [d, q-blk] += xn_chunk.T @ PT_chunk      (accumulated over 16 chunks)
  den  via DVE pair-accumulator + 2 ones-matmuls; reciprocal on DVE
  outT [dout, q-blk] += W2_h^T-style matmul over 5 heads, then DMA out

Emission is software-pipelined with a 2-tile score lookahead so the ACT
engine (the critical engine: 21M exp elements/core) never waits on PE.
"""

from collections import deque

import numpy as np

import concourse.tile as tile
from concourse import bacc, mybir
from concourse.bass import ds, ts
from concourse.bass_utils import run_bass_kernel_spmd

B, N, D, H = 4, 2048, 128, 10
HL = H // 2  # heads per core
NCHUNK = N // 128  # 16 key chunks
NBLK = N // 512  # 4 query blocks
NPAIR = NCHUNK // 2  # 8 chunk-pairs per (nb, h)
INV_SCALE = float(1.0 / (128.0**0.5 + 1e-8))
f32 = mybir.dt.float32
fp16 = mybir.dt.float16

PROFILE = False
LAST_RESULTS = None

_built = None


def _emit(tc, xT, xn, m_in, w2, ones_dram, outT):
    nc = tc.nc
    Exp = mybir.ActivationFunctionType.Exp

    from contextlib import ExitStack

    ctx = ExitStack()
    consts = ctx.enter_context(tc.tile_pool(name="consts", bufs=1))
    proj = ctx.enter_context(tc.tile_pool(name="proj", bufs=1))
    ps = ctx.enter_context(tc.tile_pool(name="ps", bufs=2, space="PSUM"))
    otps = ctx.enter_context(tc.tile_pool(name="otps", bufs=2, space="PSUM"))
    dnps = ctx.enter_context(tc.tile_pool(name="dnps", bufs=1, space="PSUM"))
    outps = ctx.enter_context(tc.tile_pool(name="outps", bufs=1, space="PSUM"))
    ptp = ctx.enter_context(tc.tile_pool(name="ptp", bufs=4))
    work = ctx.enter_context(tc.tile_pool(name="work", bufs=2))

    ones_mat = consts.tile([128, 128], fp16)
    xT_sb = consts.tile([D, N], fp16)
    xn_sb = consts.tile([D, NCHUNK, 128], fp16)  # [p, c, d] = x[c*128+p, d]
    m_sb = consts.tile([D, HL * D], fp16)
    w2_sb = consts.tile([D, HL * D], fp16)

    # --- input DMAs, spread across queues; critical tiles first ---
    nc.sync.dma_start(m_sb[:, ts(0, D)], m_in[0])
    nc.sync.dma_start(xT_sb[:, ts(0, 1024)], xT[:, ts(0, 1024)])
    nc.sync.dma_start(xT_sb[:, ts(1, 1024)], xT[:, ts(1, 1024)])
    # xn chunk loads: each is a clean [128 part, 256B] natural-layout block
    xn_src = xn.rearrange("(c p) d -> p c d", p=128)
    qs = [nc.sync, nc.gpsimd, nc.scalar]
    for c in range(NCHUNK):
        qs[c % 3].dma_start(xn_sb[:, c], xn_src[:, c])
    nc.gpsimd.dma_start(ones_mat[:], ones_dram)
    for h in range(1, HL):
        nc.sync.dma_start(m_sb[:, ts(h, D)], m_in[h])
    for h in range(HL):
        nc.gpsimd.dma_start(w2_sb[:, ts(h, D)], w2[h])

    gt = proj.tile([D, HL * N], fp16)

    # --- projection jobs: gt[:, h*N + j*512] = A_h^T-contracted block ---
    # head 0 runs upfront (gates the very first scores); heads 1..4 are
    # interleaved into head-0's chunk stream. First 4 evacs ride ScalarE
    # (idle before the first EXP); the rest ride VectorE.
    evac_ctr = [0]

    def proj_job(h, j):
        p = ps.tile([128, 2, 512], f32, tag="st", name="projp")
        nc.tensor.matmul(
            p[:, 0],
            m_sb[:, ts(h, D)],
            xT_sb[:, ts(j, 512)],
            start=True,
            stop=True,
        )
        dst = gt[:, ds(h * N + j * 512, 512)]
        if evac_ctr[0] < 4:
            nc.scalar.copy(dst, p[:, 0])
        else:
            nc.vector.tensor_copy(dst, p[:, 0])
        evac_ctr[0] += 1

    for j in range(NBLK):
        proj_job(0, j)
    proj_jobs = deque((h, j) for h in range(1, HL) for j in range(NBLK))

    # --- flat score-tile schedule with lookahead-2 production ---
    tiles = [(nb, h, cp) for nb in range(NBLK) for h in range(HL) for cp in range(NPAIR)]
    NT = len(tiles)
    st_tiles = {}

    def produce(i):
        if i >= NT:
            return
        nb, h, cp = tiles[i]
        stp = ps.tile([128, 2, 512], f32, tag="st", name=f"st{i}")
        for j in range(2):
            nc.tensor.matmul(
                stp[:, j],
                xT_sb[:, ts(2 * cp + j, 128)],
                gt[:, ds(h * N + nb * 512, 512)],
                start=True,
                stop=True,
            )
        st_tiles[i] = stp

    # per-(nb,h) state
    pend = None  # epilogue of the previous head

    def emit_finish(st):
        otn = work.tile([128, 512], fp16, tag="otn")
        nc.vector.tensor_mul(otn[:], st["ot_ps"][:], st["bc"][:])
        nc.tensor.matmul(
            st["outp"][:],
            w2_sb[:, ts(st["h"], D)],
            otn[:],
            start=(st["h"] == 0),
            stop=(st["h"] == HL - 1),
        )
        if st["h"] == HL - 1:
            osb = work.tile([128, 512], f32, tag="osb")
            nc.vector.tensor_copy(osb[:], st["outp"][:])
            nc.sync.dma_start(outT[:, ts(st["nb"], 512)], osb[:])

    PRE = 2
    for i in range(PRE):
        produce(i)

    ot_ps = acc = outp = None
    for i, (nb, h, cp) in enumerate(tiles):
        if cp == 0:
            ot_ps = otps.tile([128, 512], f32, tag="ot_ps")
            if h == 0:
                outp = outps.tile([128, 512], f32, tag="outp")
        stp = st_tiles.pop(i)
        p = ptp.tile([128, 2, 512], fp16, tag="pt")
        nc.scalar.activation(p[:], stp[:], Exp, scale=INV_SCALE)
        produce(i + PRE)
        # interleave projections for heads 1..4 into head-0's stream
        if proj_jobs and i < NPAIR:
            proj_job(*proj_jobs.popleft())
            proj_job(*proj_jobs.popleft())
        # P @ X accumulation for this pair
        for j in range(2):
            cc = 2 * cp + j
            nc.tensor.matmul(
                ot_ps[:],
                xn_sb[:, cc],
                p[:, j],
                start=(cc == 0),
                stop=(cc == NCHUNK - 1),
            )
        # denominator pair-accumulator on DVE
        if cp == 0:
            acc = work.tile([128, 2, 512], fp16, tag="dacc")
            nc.vector.tensor_copy(acc[:], p[:])
        else:
            nc.vector.tensor_add(acc[:], acc[:], p[:])
        # previous head's epilogue, mid-stream where PE has slack
        if pend is not None and cp == 4:
            emit_finish(pend)
            pend = None
        if cp == NPAIR - 1:
            dn_ps = dnps.tile([128, 512], f32, tag="dn_ps")
            for j in range(2):
                nc.tensor.matmul(
                    dn_ps[:],
                    ones_mat[:],
                    acc[:, j],
                    start=(j == 0),
                    stop=(j == 1),
                )
            bc = work.tile([128, 512], f32, tag="bc")
            nc.vector.reciprocal_approx_fast(out=bc[:], in_=dn_ps[:])
            pend = {"ot_ps": ot_ps, "bc": bc, "outp": outp, "h": h, "nb": nb}
    emit_finish(pend)
    ctx.close()


def _build():
    nc = bacc.Bacc("TRN2", target_bir_lowering=False, debug=False)
    xT = nc.dram_tensor("xT", [D, N], fp16, kind="ExternalInput").ap()
    xn = nc.dram_tensor("xn", [N, D], fp16, kind="ExternalInput").ap()
    m_in = nc.dram_tensor("m_in", [HL, D, D], fp16, kind="ExternalInput").ap()
    w2 = nc.dram_tensor("w2", [HL, D, D], fp16, kind="ExternalInput").ap()
    ones_dram = nc.dram_tensor("ones", [D, D], fp16, kind="ExternalInput").ap()
    outT = nc.dram_tensor("outT", [D, N], f32, kind="ExternalOutput").ap()
    with tile.TileContext(nc) as tc:
        with nc.allow_low_precision(reason="fp16 matmul operands"):
            _emit(tc, xT, xn, m_in, w2, ones_dram, outT)
    nc.compile()
    return nc


def kernel(x, Wq, Wk, Wv, Wo, bo):
    global _built, LAST_RESULTS
    x = np.asarray(x, dtype=np.float32)
    Wq = np.asarray(Wq, dtype=np.float32)
    Wk = np.asarray(Wk, dtype=np.float32)
    Wv = np.asarray(Wv, dtype=np.float32)
    Wo = np.asarray(Wo, dtype=np.float32)
    bo = np.asarray(bo, dtype=np.float32)

    if _built is None:
        _built = _build()
    nc = _built

    # A_h = Wq_h^T @ Wk_h ; W2_h = Wv_h^T @ Wo_h^T
    A = np.einsum("hed,hef->hdf", Wq, Wk).astype(np.float16)
    W2 = np.einsum(
        "hde,heo->hdo", Wv.transpose(0, 2, 1), Wo.T.reshape(H, D, D)
    ).astype(np.float16)
    A = np.ascontiguousarray(A)
    W2 = np.ascontiguousarray(W2)

    in_maps = []
    for c in range(8):
        b, g = divmod(c, 2)
        hsl = slice(g * HL, g * HL + HL)
        in_maps.append(
            {
                "xT": np.ascontiguousarray(x[b].T.astype(np.float16)),
                "xn": np.ascontiguousarray(x[b].astype(np.float16)),
                "m_in": A[hsl],
                "w2": W2[hsl],
                "ones": np.ones((D, D), dtype=np.float16),
            }
        )

    res = run_bass_kernel_spmd(
        nc, in_maps, core_ids=list(range(8)), trace=PROFILE
    )
    LAST_RESULTS = res

    out = np.empty((B, N, D), dtype=np.float32)
    for b in range(B):
        oT = res.results[2 * b]["outT"] + res.results[2 * b + 1]["outT"]
        out[b] = oT.T
    out += bo
    return out
